# revision 28
# baseline (speedup 1.0000x reference)
# Binary linear: y[b,s,o] = sum_i x[b,s,i] * sign(W)[o,i]
#
# Strategy (8 NeuronCores, data-parallel over tokens):
#   - Host: flatten x to [32768, 768] and shard 8 x [4096, 768]. Per core,
#     pack x per 512-token group with the contraction dim on SBUF
#     partitions, p-major so every DMA lands with multi-KB contiguous
#     partition rows (small rows halve the DMA queues' effective rate).
#     Contraction blocks k0/k1 are quantized to fp8 e4m3, k2..k5 stay bf16.
#     Weights are sign(W) (exactly +-1): fp8 for k0/k1, bf16 for k2..k5.
#   - Device (per core): out[o-block, token] layout. Per (group, out-slab):
#     four bf16 matmuls (k2..k5, N=512) plus ONE fp8 DoubleRow matmul that
#     contracts k0+k1 together at ~1.44x the bf16 rate. Out-slabs run in
#     PAIRS with the k-loop interleaved between the two PSUM banks: a
#     single bank caps the accumulate stream at ~2.0GHz, alternating banks
#     sustains the full ~2.4GHz PE rate. Evictions are DVE f32->bf16 casts
#     (kept off the scalar engine, whose DMA issues would delay them and
#     stall PSUM recycling); y stores are linear 128KB DMAs balanced across
#     the two hardware DMA queues. A short PE warmup covers the ~3us DMA
#     launch+ramp latency.
#   - Accuracy: only x carries rounding error (w is exact): fp8 on 2/6 of
#     the contraction + bf16 elsewhere + bf16 y => rel err ~1.55e-2
#     (measured), within the 2e-2 gate with margin.
#   - Host: unpack [os][g][128, 512] -> [4, 8192, 768] f32.

import numpy as np

N_CORES = 8
B, S, D_IN, D_OUT = 4, 8192, 768, 768
T_TOTAL = B * S              # 32768 tokens
T_CORE = T_TOTAL // N_CORES  # 4096 tokens per core
P = 128
KB = D_IN // P               # 6 contraction blocks (k0/k1 fp8, k2-5 bf16)
OS = D_OUT // P              # 6 out-feature slabs
TG = 512                     # tokens per group (one PSUM bank of f32)
G = T_CORE // TG             # 8 groups per core
N_WARMUP = 5

_cache = {}


def _build():
    import concourse.bacc as bacc
    import concourse.mybir as mybir
    import concourse.tile as tile

    f32 = mybir.dt.float32
    bf16 = mybir.dt.bfloat16
    fp8 = mybir.dt.float8e4
    DR = mybir.MatmulPerfMode.DoubleRow

    nc = bacc.Bacc(
        "TRN2",
        target_bir_lowering=False,
        debug=False,
        num_devices=N_CORES,
    )

    x8P = nc.dram_tensor("x8P", [G, P, 2, TG], fp8, kind="ExternalInput")
    xBP = nc.dram_tensor("xBP", [G, P, KB - 2, TG], bf16, kind="ExternalInput")
    w8P = nc.dram_tensor("w8P", [P, OS, 2, P], fp8, kind="ExternalInput")
    wBP = nc.dram_tensor("wBP", [OS, P, KB - 2, P], bf16, kind="ExternalInput")
    yP = nc.dram_tensor("yP", [OS, G, P, TG], bf16, kind="ExternalOutput")

    with tile.TileContext(nc) as tc:
        with (
            tc.tile_pool(name="wpool", bufs=1) as w_pool,
            tc.tile_pool(name="xpool", bufs=1) as x_pool,
            tc.tile_pool(name="ypool", bufs=8) as y_pool,
            tc.tile_pool(name="psum", bufs=6, space="PSUM") as psum_pool,
        ):
            # --- PE warmup: dummy matmuls on zeroed scratch so the PE clock
            # has ramped by the time the first real operands land. ---
            wu = x_pool.tile([P, P + TG], bf16, tag="wu", name="wu", bufs=1)
            nc.gpsimd.memset(wu[:], 0.0)
            wups = psum_pool.tile([P, TG], f32, tag="wups", name="wups", bufs=1)
            for _ in range(N_WARMUP):
                nc.tensor.matmul(
                    wups[:], wu[:, :P], wu[:, P:],
                    start=True, stop=True, skip_group_check=True,
                )
            wu_out = x_pool.tile([P, TG], bf16, tag="wuo", name="wuo", bufs=1)
            nc.vector.tensor_copy(wu_out[:], wups[:])

            # --- loads, interleaved across the two HW queues in need order
            # (the per-(g,os) k-order is bf16 k2..k5 first, fp8 DR last, so
            # the fp8 operands may arrive ~1.7us later than the bf16 ones)
            w8all = w_pool.tile([P, OS, 2, P], fp8, tag="w8", name="w8")
            wB = [None] * OS

            def wB_load(os_, eng):
                t = w_pool.tile([P, KB - 2, P], bf16, tag=f"wB{os_}", name=f"wB{os_}")
                eng.dma_start(t[:], wBP[os_])
                wB[os_] = t

            x8 = [None] * G

            def x8_load(g, eng):
                t = x_pool.tile([P, 2, TG], fp8, tag=f"x8_{g}", name=f"x8_{g}")
                eng.dma_start(t[:], x8P[g])
                x8[g] = t

            xB = [None] * G
            xB0h = [None, None]

            def xB0_load(h, eng):
                # group 0 bf16 x as two k-pair tiles so the queues can fill
                # them in parallel just ahead of the PE
                t = x_pool.tile([P, 2, TG], bf16, tag=f"xB0_{h}", name=f"xB0_{h}")
                eng.dma_start(t[:], xBP[0, :, 2 * h : 2 * h + 2, :])
                xB0h[h] = t

            def xB_load(g, eng):
                t = x_pool.tile([P, KB - 2, TG], bf16, tag=f"xB{g}", name=f"xB{g}")
                eng.dma_start(t[:], xBP[g])
                xB[g] = t

            wB_load(0, nc.sync)
            nc.scalar.dma_start(w8all[:], w8P[:])
            xB0_load(0, nc.sync)
            wB_load(1, nc.scalar)
            x8_load(0, nc.sync)
            xB0_load(1, nc.scalar)
            wB_load(2, nc.sync)
            wB_load(3, nc.scalar)
            xB_load(1, nc.sync)
            wB_load(4, nc.scalar)
            wB_load(5, nc.scalar)
            x8_load(1, nc.scalar)
            xB_load(3, nc.sync)
            xB_load(2, nc.scalar)
            x8_load(2, nc.scalar)
            x8_load(3, nc.scalar)
            xB_load(5, nc.sync)
            x8_load(4, nc.scalar)
            x8_load(5, nc.scalar)
            xB_load(6, nc.sync)
            x8_load(6, nc.scalar)
            x8_load(7, nc.scalar)
            xB_load(7, nc.sync)
            xB_load(4, nc.scalar)

            def rhsB(g, k):
                if g == 0:
                    return xB0h[(k - 2) // 2][:, (k - 2) % 2, :]
                return xB[g][:, k - 2, :]

            # --- main loop: out-slab pairs, k-loop interleaved across the
            # pair's two PSUM banks to sustain the full PE rate; bf16 k2..k5
            # first, then one fp8 DoubleRow matmul contracting k0+k1 ---
            ecnt = 0
            for g in range(G):
                for osp in range(OS // 2):
                    os_a, os_b = 2 * osp, 2 * osp + 1
                    ps_a = psum_pool.tile([P, TG], f32, tag="ps", name=f"ps{g}_{os_a}")
                    ps_b = psum_pool.tile([P, TG], f32, tag="ps", name=f"ps{g}_{os_b}")
                    for k in range(2, KB):
                        st = k == 2
                        nc.tensor.matmul(
                            ps_a[:], wB[os_a][:, k - 2, :], rhsB(g, k),
                            start=st, stop=False,
                        )
                        nc.tensor.matmul(
                            ps_b[:], wB[os_b][:, k - 2, :], rhsB(g, k),
                            start=st, stop=False,
                        )
                    rhs8 = x8[g][:]
                    nc.tensor.matmul(
                        ps_a[:], w8all[:, os_a, :, :], rhs8,
                        start=False, stop=True, perf_mode=DR,
                    )
                    nc.tensor.matmul(
                        ps_b[:], w8all[:, os_b, :, :], rhs8,
                        start=False, stop=True, perf_mode=DR,
                    )
                    for os_, ps in ((os_a, ps_a), (os_b, ps_b)):
                        yt = y_pool.tile([P, TG], bf16, tag="y", name=f"y{g}_{os_}")
                        if g == G - 1 and osp == OS // 2 - 1:
                            # tail pair: halves in parallel on both copy
                            # engines and both DMA queues so the final
                            # receipts land as early as possible
                            h = TG // 2
                            nc.vector.tensor_copy(yt[:, :h], ps[:, :h])
                            nc.scalar.copy(yt[:, h:], ps[:, h:])
                            nc.sync.dma_start(yP[os_, g, :, :h], yt[:, :h])
                            nc.scalar.dma_start(yP[os_, g, :, h:], yt[:, h:])
                            ecnt += 1
                            continue
                        # all evictions on DVE: the scalar engine's DMA
                        # issues would delay them and stall PSUM recycling
                        nc.vector.tensor_copy(yt[:], ps[:])
                        # store queues: scalar while sync still streams x,
                        # alternating afterwards so neither queue backs up
                        # near the tail
                        if g <= 4:
                            q = nc.scalar
                        else:
                            q = nc.sync if os_ % 2 == 0 else nc.scalar
                        q.dma_start(yP[os_, g], yt[:])
                        ecnt += 1

    nc.compile()
    return nc


def _get_nc():
    if "nc" not in _cache:
        _cache["nc"] = _build()
    return _cache["nc"]


def _prep_inputs(x, weight):
    import ml_dtypes

    x = np.asarray(x, dtype=np.float32)
    w = np.asarray(weight, dtype=np.float32)
    x2 = x.reshape(N_CORES, T_CORE, D_IN)
    # x5[c, g, t, k, p] = x2[c, g*TG + t, k*P + p] -> packs [c, g, p, k, t]
    x5 = x2.reshape(N_CORES, G, TG, KB, P)
    x8Pack = np.ascontiguousarray(x5[:, :, :, :2, :].transpose(0, 1, 4, 3, 2)).astype(
        ml_dtypes.float8_e4m3fn
    )
    xBPack = np.ascontiguousarray(x5[:, :, :, 2:, :].transpose(0, 1, 4, 3, 2)).astype(
        ml_dtypes.bfloat16
    )
    # S4[os, o, k, p] = sign(W)[os*P + o, k*P + p]  (+-1/0 exact in both)
    S4 = np.sign(w).reshape(OS, P, KB, P)
    w8Pack = np.ascontiguousarray(S4[:, :, :2, :].transpose(3, 0, 2, 1)).astype(
        ml_dtypes.float8_e4m3fn
    )
    wBPack = np.ascontiguousarray(S4[:, :, 2:, :].transpose(0, 3, 2, 1)).astype(
        ml_dtypes.bfloat16
    )
    return [
        {"x8P": x8Pack[c], "xBP": xBPack[c], "w8P": w8Pack, "wBP": wBPack}
        for c in range(N_CORES)
    ]


def _unpack_output(res):
    # yP [OS, G, P(o), TG(t)] -> y_core [T_CORE, D_OUT]
    outs = []
    for r in res.results:
        yp = np.asarray(r["yP"]).astype(np.float32)
        outs.append(yp.transpose(1, 3, 0, 2).reshape(T_CORE, D_OUT))
    return np.concatenate(outs, axis=0).reshape(B, S, D_OUT)


def _install_axon_ntff_hook():
    """The agent image's `antenv` lacks `axon_hooks`; register an equivalent
    module backed by direct ctypes calls into libaxon_pjrt.so so that
    run_bass_kernel_spmd(trace=True) can capture NTFF profiles under axon."""
    import sys

    if "antenv.axon_hooks" in sys.modules:
        return
    import contextlib
    import ctypes
    import types

    so_path = "/opt/axon/libaxon_pjrt.so"
    try:
        lib = ctypes.CDLL(so_path)
    except OSError:
        return
    if not hasattr(lib, "axon_start_nrt_profile"):
        return
    lib.axon_start_nrt_profile.argtypes = [
        ctypes.POINTER(ctypes.c_int64),
        ctypes.c_size_t,
    ]
    lib.axon_start_nrt_profile.restype = ctypes.c_int64
    lib.axon_stop_nrt_profile.argtypes = [ctypes.c_char_p]
    lib.axon_stop_nrt_profile.restype = ctypes.c_int64

    @contextlib.contextmanager
    def _hook(output_dir, device_ids):
        import jax

        jax.devices()
        if device_ids:
            ids = (ctypes.c_int64 * len(device_ids))(*device_ids)
            rc = lib.axon_start_nrt_profile(ids, len(device_ids))
        else:
            rc = lib.axon_start_nrt_profile(None, 0)
        if rc != 0:
            raise RuntimeError(f"axon_start_nrt_profile rc={rc}")
        try:
            yield
        finally:
            n = lib.axon_stop_nrt_profile(str(output_dir).encode())
            print(f"ntff profile: {n} file(s) written to {output_dir}")

    mod = types.ModuleType("antenv.axon_hooks")
    mod.get_axon_ntff_profile_hook = lambda: _hook
    mod.set_axon_ntff_profile_hook = lambda h: None
    sys.modules["antenv.axon_hooks"] = mod


def _run(x, weight, trace=False):
    from concourse.bass_utils import run_bass_kernel_spmd

    if trace:
        _install_axon_ntff_hook()
    nc = _get_nc()
    in_maps = _prep_inputs(x, weight)
    res = run_bass_kernel_spmd(
        nc, in_maps, core_ids=list(range(N_CORES)), trace=trace
    )
    return _unpack_output(res), res


def kernel(x, weight):
    out, _ = _run(x, weight, trace=False)
    return out


# revision 32
# speedup vs baseline: 1.0084x; 1.0084x over previous
# Binary linear: y[b,s,o] = sum_i x[b,s,i] * sign(W)[o,i]
#
# Strategy (8 NeuronCores, data-parallel over tokens):
#   - Host: flatten x to [32768, 768] and shard 8 x [4096, 768]. Per core,
#     pack x per 512-token group with the contraction dim on SBUF
#     partitions, p-major so every DMA lands with multi-KB contiguous
#     partition rows (small rows halve the DMA queues' effective rate).
#     Contraction blocks k0/k1 are quantized to fp8 e4m3, k2..k5 stay bf16.
#     Weights are sign(W) (exactly +-1): fp8 for k0/k1, bf16 for k2..k5.
#   - Device (per core): out[o-block, token] layout. Per (group, out-slab):
#     four bf16 matmuls (k2..k5, N=512) plus ONE fp8 DoubleRow matmul that
#     contracts k0+k1 together at ~1.44x the bf16 rate. Out-slabs run in
#     PAIRS with the k-loop interleaved between the two PSUM banks: a
#     single bank caps the accumulate stream at ~2.0GHz, alternating banks
#     sustains the full ~2.4GHz PE rate. Evictions are DVE f32->bf16 casts
#     (kept off the scalar engine, whose DMA issues would delay them and
#     stall PSUM recycling); y stores are linear 128KB DMAs balanced across
#     the two hardware DMA queues. A short PE warmup covers the ~3us DMA
#     launch+ramp latency.
#   - Accuracy: only x carries rounding error (w is exact): fp8 on 2/6 of
#     the contraction + bf16 elsewhere + bf16 y => rel err ~1.55e-2
#     (measured), within the 2e-2 gate with margin.
#   - Host: unpack [os][g][128, 512] -> [4, 8192, 768] f32.

import numpy as np

N_CORES = 8
B, S, D_IN, D_OUT = 4, 8192, 768, 768
T_TOTAL = B * S              # 32768 tokens
T_CORE = T_TOTAL // N_CORES  # 4096 tokens per core
P = 128
KB = D_IN // P               # 6 contraction blocks (k0/k1 fp8, k2-5 bf16)
OS = D_OUT // P              # 6 out-feature slabs
TG = 512                     # tokens per group (one PSUM bank of f32)
G = T_CORE // TG             # 8 groups per core
N_WARMUP = 5

_cache = {}


def _build():
    import concourse.bacc as bacc
    import concourse.mybir as mybir
    import concourse.tile as tile

    f32 = mybir.dt.float32
    bf16 = mybir.dt.bfloat16
    fp8 = mybir.dt.float8e4
    DR = mybir.MatmulPerfMode.DoubleRow

    nc = bacc.Bacc(
        "TRN2",
        target_bir_lowering=False,
        debug=False,
        num_devices=N_CORES,
    )

    # fp8 x pair-interleaved per token ([t, pair] innermost) so the DR
    # moving stream presents both pair values adjacently per column
    x8P = nc.dram_tensor("x8P", [G, P, TG, 2], fp8, kind="ExternalInput")
    xBP = nc.dram_tensor("xBP", [G, P, KB - 2, TG], bf16, kind="ExternalInput")
    w8P = nc.dram_tensor("w8P", [P, OS, 2, P], fp8, kind="ExternalInput")
    wBP = nc.dram_tensor("wBP", [OS, P, KB - 2, P], bf16, kind="ExternalInput")
    yP = nc.dram_tensor("yP", [OS, G, P, TG], bf16, kind="ExternalOutput")

    with tile.TileContext(nc) as tc:
        with (
            tc.tile_pool(name="wpool", bufs=1) as w_pool,
            tc.tile_pool(name="xpool", bufs=1) as x_pool,
            tc.tile_pool(name="ypool", bufs=8) as y_pool,
            tc.tile_pool(name="psum", bufs=6, space="PSUM") as psum_pool,
        ):
            # --- PE warmup: dummy matmuls on zeroed scratch so the PE clock
            # has ramped by the time the first real operands land. ---
            wu = x_pool.tile([P, P + TG], bf16, tag="wu", name="wu", bufs=1)
            nc.gpsimd.memset(wu[:], 0.0)
            wups = psum_pool.tile([P, TG], f32, tag="wups", name="wups", bufs=1)
            for _ in range(N_WARMUP):
                nc.tensor.matmul(
                    wups[:], wu[:, :P], wu[:, P:],
                    start=True, stop=True, skip_group_check=True,
                )
            wu_out = x_pool.tile([P, TG], bf16, tag="wuo", name="wuo", bufs=1)
            nc.vector.tensor_copy(wu_out[:], wups[:])

            # --- loads, interleaved across the two HW queues in need order
            # (the per-(g,os) k-order is bf16 k2..k5 first, fp8 DR last, so
            # the fp8 operands may arrive ~1.7us later than the bf16 ones)
            w8all = w_pool.tile([P, OS, 2, P], fp8, tag="w8", name="w8")
            wB = [None] * OS

            def wB_load(os_, eng):
                t = w_pool.tile([P, KB - 2, P], bf16, tag=f"wB{os_}", name=f"wB{os_}")
                eng.dma_start(t[:], wBP[os_])
                wB[os_] = t

            x8 = [None] * G

            def x8_load(g, eng):
                t = x_pool.tile([P, TG, 2], fp8, tag=f"x8_{g}", name=f"x8_{g}")
                eng.dma_start(t[:], x8P[g])
                x8[g] = t

            xB = [None] * G
            xB0h = [None, None]

            def xB0_load(h, eng):
                # group 0 bf16 x as two k-pair tiles so the queues can fill
                # them in parallel just ahead of the PE
                t = x_pool.tile([P, 2, TG], bf16, tag=f"xB0_{h}", name=f"xB0_{h}")
                eng.dma_start(t[:], xBP[0, :, 2 * h : 2 * h + 2, :])
                xB0h[h] = t

            def xB_load(g, eng):
                t = x_pool.tile([P, KB - 2, TG], bf16, tag=f"xB{g}", name=f"xB{g}")
                eng.dma_start(t[:], xBP[g])
                xB[g] = t

            wB_load(0, nc.sync)
            nc.scalar.dma_start(w8all[:], w8P[:])
            xB0_load(0, nc.sync)
            wB_load(1, nc.scalar)
            x8_load(0, nc.sync)
            xB0_load(1, nc.scalar)
            wB_load(2, nc.sync)
            wB_load(3, nc.scalar)
            xB_load(1, nc.sync)
            wB_load(4, nc.scalar)
            wB_load(5, nc.scalar)
            x8_load(1, nc.scalar)
            xB_load(3, nc.sync)
            xB_load(2, nc.scalar)
            x8_load(2, nc.scalar)
            x8_load(3, nc.scalar)
            xB_load(5, nc.sync)
            x8_load(4, nc.scalar)
            x8_load(5, nc.scalar)
            xB_load(6, nc.sync)
            x8_load(6, nc.scalar)
            x8_load(7, nc.scalar)
            xB_load(7, nc.sync)
            xB_load(4, nc.scalar)

            def rhsB(g, k):
                if g == 0:
                    return xB0h[(k - 2) // 2][:, (k - 2) % 2, :]
                return xB[g][:, k - 2, :]

            # --- main loop: out-slab pairs, k-loop interleaved across the
            # pair's two PSUM banks to sustain the full PE rate; bf16 k2..k5
            # first, then one fp8 DoubleRow matmul contracting k0+k1 ---
            ecnt = 0
            for g in range(G):
                for osp in range(OS // 2):
                    os_a, os_b = 2 * osp, 2 * osp + 1
                    ps_a = psum_pool.tile([P, TG], f32, tag="ps", name=f"ps{g}_{os_a}")
                    ps_b = psum_pool.tile([P, TG], f32, tag="ps", name=f"ps{g}_{os_b}")
                    for k in range(2, KB):
                        st = k == 2
                        nc.tensor.matmul(
                            ps_a[:], wB[os_a][:, k - 2, :], rhsB(g, k),
                            start=st, stop=False,
                        )
                        nc.tensor.matmul(
                            ps_b[:], wB[os_b][:, k - 2, :], rhsB(g, k),
                            start=st, stop=False,
                        )
                    rhs8 = x8[g][:].rearrange("p t two -> p two t")
                    nc.tensor.matmul(
                        ps_a[:], w8all[:, os_a, :, :], rhs8,
                        start=False, stop=True, perf_mode=DR,
                    )
                    nc.tensor.matmul(
                        ps_b[:], w8all[:, os_b, :, :], rhs8,
                        start=False, stop=True, perf_mode=DR,
                    )
                    for os_, ps in ((os_a, ps_a), (os_b, ps_b)):
                        yt = y_pool.tile([P, TG], bf16, tag="y", name=f"y{g}_{os_}")
                        if g == G - 1 and osp == OS // 2 - 1:
                            # tail pair: halves in parallel on both copy
                            # engines and both DMA queues so the final
                            # receipts land as early as possible
                            h = TG // 2
                            nc.vector.tensor_copy(yt[:, :h], ps[:, :h])
                            nc.scalar.copy(yt[:, h:], ps[:, h:])
                            nc.sync.dma_start(yP[os_, g, :, :h], yt[:, :h])
                            nc.scalar.dma_start(yP[os_, g, :, h:], yt[:, h:])
                            ecnt += 1
                            continue
                        # all evictions on DVE: the scalar engine's DMA
                        # issues would delay them and stall PSUM recycling
                        nc.vector.tensor_copy(yt[:], ps[:])
                        # store queues: scalar while sync still streams x,
                        # alternating afterwards so neither queue backs up
                        # near the tail
                        if g <= 4:
                            q = nc.scalar
                        else:
                            q = nc.sync if os_ % 2 == 0 else nc.scalar
                        q.dma_start(yP[os_, g], yt[:])
                        ecnt += 1

    nc.compile()
    return nc


def _get_nc():
    if "nc" not in _cache:
        _cache["nc"] = _build()
    return _cache["nc"]


def _prep_inputs(x, weight):
    import ml_dtypes

    x = np.asarray(x, dtype=np.float32)
    w = np.asarray(weight, dtype=np.float32)
    x2 = x.reshape(N_CORES, T_CORE, D_IN)
    # x5[c, g, t, k, p] = x2[c, g*TG + t, k*P + p] -> packs [c, g, p, k, t]
    x5 = x2.reshape(N_CORES, G, TG, KB, P)
    # [c, g, p, t, pair]: both pair values adjacent per token column
    x8Pack = np.ascontiguousarray(x5[:, :, :, :2, :].transpose(0, 1, 4, 2, 3)).astype(
        ml_dtypes.float8_e4m3fn
    )
    xBPack = np.ascontiguousarray(x5[:, :, :, 2:, :].transpose(0, 1, 4, 3, 2)).astype(
        ml_dtypes.bfloat16
    )
    # S4[os, o, k, p] = sign(W)[os*P + o, k*P + p]  (+-1/0 exact in both)
    S4 = np.sign(w).reshape(OS, P, KB, P)
    w8Pack = np.ascontiguousarray(S4[:, :, :2, :].transpose(3, 0, 2, 1)).astype(
        ml_dtypes.float8_e4m3fn
    )
    wBPack = np.ascontiguousarray(S4[:, :, 2:, :].transpose(0, 3, 2, 1)).astype(
        ml_dtypes.bfloat16
    )
    return [
        {"x8P": x8Pack[c], "xBP": xBPack[c], "w8P": w8Pack, "wBP": wBPack}
        for c in range(N_CORES)
    ]


def _unpack_output(res):
    # yP [OS, G, P(o), TG(t)] -> y_core [T_CORE, D_OUT]
    outs = []
    for r in res.results:
        yp = np.asarray(r["yP"]).astype(np.float32)
        outs.append(yp.transpose(1, 3, 0, 2).reshape(T_CORE, D_OUT))
    return np.concatenate(outs, axis=0).reshape(B, S, D_OUT)


def _install_axon_ntff_hook():
    """The agent image's `antenv` lacks `axon_hooks`; register an equivalent
    module backed by direct ctypes calls into libaxon_pjrt.so so that
    run_bass_kernel_spmd(trace=True) can capture NTFF profiles under axon."""
    import sys

    if "antenv.axon_hooks" in sys.modules:
        return
    import contextlib
    import ctypes
    import types

    so_path = "/opt/axon/libaxon_pjrt.so"
    try:
        lib = ctypes.CDLL(so_path)
    except OSError:
        return
    if not hasattr(lib, "axon_start_nrt_profile"):
        return
    lib.axon_start_nrt_profile.argtypes = [
        ctypes.POINTER(ctypes.c_int64),
        ctypes.c_size_t,
    ]
    lib.axon_start_nrt_profile.restype = ctypes.c_int64
    lib.axon_stop_nrt_profile.argtypes = [ctypes.c_char_p]
    lib.axon_stop_nrt_profile.restype = ctypes.c_int64

    @contextlib.contextmanager
    def _hook(output_dir, device_ids):
        import jax

        jax.devices()
        if device_ids:
            ids = (ctypes.c_int64 * len(device_ids))(*device_ids)
            rc = lib.axon_start_nrt_profile(ids, len(device_ids))
        else:
            rc = lib.axon_start_nrt_profile(None, 0)
        if rc != 0:
            raise RuntimeError(f"axon_start_nrt_profile rc={rc}")
        try:
            yield
        finally:
            n = lib.axon_stop_nrt_profile(str(output_dir).encode())
            print(f"ntff profile: {n} file(s) written to {output_dir}")

    mod = types.ModuleType("antenv.axon_hooks")
    mod.get_axon_ntff_profile_hook = lambda: _hook
    mod.set_axon_ntff_profile_hook = lambda h: None
    sys.modules["antenv.axon_hooks"] = mod


def _run(x, weight, trace=False):
    from concourse.bass_utils import run_bass_kernel_spmd

    if trace:
        _install_axon_ntff_hook()
    nc = _get_nc()
    in_maps = _prep_inputs(x, weight)
    res = run_bass_kernel_spmd(
        nc, in_maps, core_ids=list(range(N_CORES)), trace=trace
    )
    return _unpack_output(res), res


def kernel(x, weight):
    out, _ = _run(x, weight, trace=False)
    return out


# revision 34
# speedup vs baseline: 1.1322x; 1.1227x over previous
# Binary linear: y[b,s,o] = sum_i x[b,s,i] * sign(W)[o,i]
#
# Strategy (8 NeuronCores, data-parallel over tokens):
#   - Host: flatten x to [32768, 768] and shard 8 x [4096, 768]. Per core,
#     pack x per 512-token group with the contraction dim on SBUF
#     partitions, p-major so every DMA lands with multi-KB contiguous
#     partition rows (small rows halve the DMA queues' effective rate).
#     Contraction blocks k0/k1 are quantized to fp8 e4m3, k2..k5 stay bf16.
#     Weights are sign(W) (exactly +-1): fp8 for k0/k1, bf16 for k2..k5.
#   - Device (per core): out[o-block, token] layout. Per (group, out-slab):
#     four bf16 matmuls (k2..k5, N=512) plus ONE fp8 DoubleRow matmul that
#     contracts k0+k1 together at ~1.44x the bf16 rate. Out-slabs run in
#     PAIRS with the k-loop interleaved between the two PSUM banks: a
#     single bank caps the accumulate stream at ~2.0GHz, alternating banks
#     sustains the full ~2.4GHz PE rate. Evictions are DVE f32->bf16 casts
#     (kept off the scalar engine, whose DMA issues would delay them and
#     stall PSUM recycling); y stores are linear 128KB DMAs balanced across
#     the two hardware DMA queues. A short PE warmup covers the ~3us DMA
#     launch+ramp latency.
#   - Accuracy: only x carries rounding error (w is exact): fp8 on 2/6 of
#     the contraction + bf16 elsewhere + bf16 y => rel err ~1.55e-2
#     (measured), within the 2e-2 gate with margin.
#   - Host: unpack [os][g][128, 512] -> [4, 8192, 768] f32.

import numpy as np

N_CORES = 8
B, S, D_IN, D_OUT = 4, 8192, 768, 768
T_TOTAL = B * S              # 32768 tokens
T_CORE = T_TOTAL // N_CORES  # 4096 tokens per core
P = 128
KB = D_IN // P               # 6 contraction blocks (k0/k1 fp8, k2-5 bf16)
OS = D_OUT // P              # 6 out-feature slabs
TG = 512                     # tokens per group (one PSUM bank of f32)
G = T_CORE // TG             # 8 groups per core
N_WARMUP = 5

_cache = {}


def _build():
    import concourse.bacc as bacc
    import concourse.mybir as mybir
    import concourse.tile as tile

    f32 = mybir.dt.float32
    bf16 = mybir.dt.bfloat16
    fp8 = mybir.dt.float8e4
    DR = mybir.MatmulPerfMode.DoubleRowSwInterleave

    nc = bacc.Bacc(
        "TRN2",
        target_bir_lowering=False,
        debug=False,
        num_devices=N_CORES,
    )

    # fp8 x pair-interleaved per token ([t, pair] innermost) so the DR
    # moving stream presents both pair values adjacently per column
    x8P = nc.dram_tensor("x8P", [G, P, TG, 2], fp8, kind="ExternalInput")
    xBP = nc.dram_tensor("xBP", [G, P, KB - 2, TG], bf16, kind="ExternalInput")
    w8P = nc.dram_tensor("w8P", [P, OS, 2, P], fp8, kind="ExternalInput")
    wBP = nc.dram_tensor("wBP", [OS, P, KB - 2, P], bf16, kind="ExternalInput")
    yP = nc.dram_tensor("yP", [OS, G, P, TG], bf16, kind="ExternalOutput")

    with tile.TileContext(nc) as tc:
        with (
            tc.tile_pool(name="wpool", bufs=1) as w_pool,
            tc.tile_pool(name="xpool", bufs=1) as x_pool,
            tc.tile_pool(name="ypool", bufs=8) as y_pool,
            tc.tile_pool(name="psum", bufs=6, space="PSUM") as psum_pool,
        ):
            # --- PE warmup: dummy matmuls on zeroed scratch so the PE clock
            # has ramped by the time the first real operands land. ---
            wu = x_pool.tile([P, P + TG], bf16, tag="wu", name="wu", bufs=1)
            nc.gpsimd.memset(wu[:], 0.0)
            wups = psum_pool.tile([P, TG], f32, tag="wups", name="wups", bufs=1)
            for _ in range(N_WARMUP):
                nc.tensor.matmul(
                    wups[:], wu[:, :P], wu[:, P:],
                    start=True, stop=True, skip_group_check=True,
                )
            wu_out = x_pool.tile([P, TG], bf16, tag="wuo", name="wuo", bufs=1)
            nc.vector.tensor_copy(wu_out[:], wups[:])

            # --- loads, interleaved across the two HW queues in need order
            # (the per-(g,os) k-order is bf16 k2..k5 first, fp8 DR last, so
            # the fp8 operands may arrive ~1.7us later than the bf16 ones)
            w8all = w_pool.tile([P, OS, 2, P], fp8, tag="w8", name="w8")
            wB = [None] * OS

            def wB_load(os_, eng):
                t = w_pool.tile([P, KB - 2, P], bf16, tag=f"wB{os_}", name=f"wB{os_}")
                eng.dma_start(t[:], wBP[os_])
                wB[os_] = t

            x8 = [None] * G

            def x8_load(g, eng):
                t = x_pool.tile([P, TG, 2], fp8, tag=f"x8_{g}", name=f"x8_{g}")
                eng.dma_start(t[:], x8P[g])
                x8[g] = t

            xB = [None] * G
            xB0h = [None, None]

            def xB0_load(h, eng):
                # group 0 bf16 x as two k-pair tiles so the queues can fill
                # them in parallel just ahead of the PE
                t = x_pool.tile([P, 2, TG], bf16, tag=f"xB0_{h}", name=f"xB0_{h}")
                eng.dma_start(t[:], xBP[0, :, 2 * h : 2 * h + 2, :])
                xB0h[h] = t

            def xB_load(g, eng):
                t = x_pool.tile([P, KB - 2, TG], bf16, tag=f"xB{g}", name=f"xB{g}")
                eng.dma_start(t[:], xBP[g])
                xB[g] = t

            wB_load(0, nc.sync)
            nc.scalar.dma_start(w8all[:], w8P[:])
            xB0_load(0, nc.sync)
            wB_load(1, nc.scalar)
            x8_load(0, nc.sync)
            xB0_load(1, nc.scalar)
            wB_load(2, nc.sync)
            wB_load(3, nc.scalar)
            xB_load(1, nc.sync)
            wB_load(4, nc.scalar)
            wB_load(5, nc.scalar)
            x8_load(1, nc.scalar)
            xB_load(3, nc.sync)
            xB_load(2, nc.scalar)
            x8_load(2, nc.scalar)
            x8_load(3, nc.scalar)
            xB_load(5, nc.sync)
            x8_load(4, nc.scalar)
            x8_load(5, nc.scalar)
            xB_load(6, nc.sync)
            x8_load(6, nc.scalar)
            x8_load(7, nc.scalar)
            xB_load(7, nc.sync)
            xB_load(4, nc.scalar)

            def rhsB(g, k):
                if g == 0:
                    return xB0h[(k - 2) // 2][:, (k - 2) % 2, :]
                return xB[g][:, k - 2, :]

            # --- main loop: out-slab pairs, k-loop interleaved across the
            # pair's two PSUM banks to sustain the full PE rate; bf16 k2..k5
            # first, then one fp8 DoubleRow matmul contracting k0+k1 ---
            ecnt = 0
            for g in range(G):
                for osp in range(OS // 2):
                    os_a, os_b = 2 * osp, 2 * osp + 1
                    ps_a = psum_pool.tile([P, TG], f32, tag="ps", name=f"ps{g}_{os_a}")
                    ps_b = psum_pool.tile([P, TG], f32, tag="ps", name=f"ps{g}_{os_b}")
                    for k in range(2, KB):
                        st = k == 2
                        nc.tensor.matmul(
                            ps_a[:], wB[os_a][:, k - 2, :], rhsB(g, k),
                            start=st, stop=False,
                        )
                        nc.tensor.matmul(
                            ps_b[:], wB[os_b][:, k - 2, :], rhsB(g, k),
                            start=st, stop=False,
                        )
                    rhs8 = x8[g][:].rearrange("p t two -> p two t")
                    nc.tensor.matmul(
                        ps_a[:], w8all[:, os_a, :, :], rhs8,
                        start=False, stop=True, perf_mode=DR,
                    )
                    nc.tensor.matmul(
                        ps_b[:], w8all[:, os_b, :, :], rhs8,
                        start=False, stop=True, perf_mode=DR,
                    )
                    for os_, ps in ((os_a, ps_a), (os_b, ps_b)):
                        yt = y_pool.tile([P, TG], bf16, tag="y", name=f"y{g}_{os_}")
                        if g == G - 1 and osp == OS // 2 - 1:
                            # tail pair: halves in parallel on both copy
                            # engines and both DMA queues so the final
                            # receipts land as early as possible
                            h = TG // 2
                            nc.vector.tensor_copy(yt[:, :h], ps[:, :h])
                            nc.scalar.copy(yt[:, h:], ps[:, h:])
                            nc.sync.dma_start(yP[os_, g, :, :h], yt[:, :h])
                            nc.scalar.dma_start(yP[os_, g, :, h:], yt[:, h:])
                            ecnt += 1
                            continue
                        # all evictions on DVE: the scalar engine's DMA
                        # issues would delay them and stall PSUM recycling
                        nc.vector.tensor_copy(yt[:], ps[:])
                        # store queues: scalar while sync still streams x,
                        # alternating afterwards so neither queue backs up
                        # near the tail
                        if g <= 4:
                            q = nc.scalar
                        else:
                            q = nc.sync if os_ % 2 == 0 else nc.scalar
                        q.dma_start(yP[os_, g], yt[:])
                        ecnt += 1

    nc.compile()
    return nc


def _get_nc():
    if "nc" not in _cache:
        _cache["nc"] = _build()
    return _cache["nc"]


def _prep_inputs(x, weight):
    import ml_dtypes

    x = np.asarray(x, dtype=np.float32)
    w = np.asarray(weight, dtype=np.float32)
    x2 = x.reshape(N_CORES, T_CORE, D_IN)
    # x5[c, g, t, k, p] = x2[c, g*TG + t, k*P + p] -> packs [c, g, p, k, t]
    x5 = x2.reshape(N_CORES, G, TG, KB, P)
    # [c, g, p, t, pair]: both pair values adjacent per token column
    x8Pack = np.ascontiguousarray(x5[:, :, :, :2, :].transpose(0, 1, 4, 2, 3)).astype(
        ml_dtypes.float8_e4m3fn
    )
    xBPack = np.ascontiguousarray(x5[:, :, :, 2:, :].transpose(0, 1, 4, 3, 2)).astype(
        ml_dtypes.bfloat16
    )
    # S4[os, o, k, p] = sign(W)[os*P + o, k*P + p]  (+-1/0 exact in both)
    S4 = np.sign(w).reshape(OS, P, KB, P)
    # SwInterleave weight layout: per partition row the k0/k1 weights are
    # interleaved pairwise with output columns reversed:
    # [A127, B127, A126, B126, ..., A0, B0]  (A=k0, B=k1)
    w8i = S4[:, ::-1, :2, :].transpose(3, 0, 1, 2)  # [p, os, o_rev, k(2)]
    w8Pack = np.ascontiguousarray(
        w8i.reshape(P, OS, 2, P)
    ).astype(ml_dtypes.float8_e4m3fn)
    wBPack = np.ascontiguousarray(S4[:, :, 2:, :].transpose(0, 3, 2, 1)).astype(
        ml_dtypes.bfloat16
    )
    return [
        {"x8P": x8Pack[c], "xBP": xBPack[c], "w8P": w8Pack, "wBP": wBPack}
        for c in range(N_CORES)
    ]


def _unpack_output(res):
    # yP [OS, G, P(o), TG(t)] -> y_core [T_CORE, D_OUT]
    outs = []
    for r in res.results:
        yp = np.asarray(r["yP"]).astype(np.float32)
        outs.append(yp.transpose(1, 3, 0, 2).reshape(T_CORE, D_OUT))
    return np.concatenate(outs, axis=0).reshape(B, S, D_OUT)


def _install_axon_ntff_hook():
    """The agent image's `antenv` lacks `axon_hooks`; register an equivalent
    module backed by direct ctypes calls into libaxon_pjrt.so so that
    run_bass_kernel_spmd(trace=True) can capture NTFF profiles under axon."""
    import sys

    if "antenv.axon_hooks" in sys.modules:
        return
    import contextlib
    import ctypes
    import types

    so_path = "/opt/axon/libaxon_pjrt.so"
    try:
        lib = ctypes.CDLL(so_path)
    except OSError:
        return
    if not hasattr(lib, "axon_start_nrt_profile"):
        return
    lib.axon_start_nrt_profile.argtypes = [
        ctypes.POINTER(ctypes.c_int64),
        ctypes.c_size_t,
    ]
    lib.axon_start_nrt_profile.restype = ctypes.c_int64
    lib.axon_stop_nrt_profile.argtypes = [ctypes.c_char_p]
    lib.axon_stop_nrt_profile.restype = ctypes.c_int64

    @contextlib.contextmanager
    def _hook(output_dir, device_ids):
        import jax

        jax.devices()
        if device_ids:
            ids = (ctypes.c_int64 * len(device_ids))(*device_ids)
            rc = lib.axon_start_nrt_profile(ids, len(device_ids))
        else:
            rc = lib.axon_start_nrt_profile(None, 0)
        if rc != 0:
            raise RuntimeError(f"axon_start_nrt_profile rc={rc}")
        try:
            yield
        finally:
            n = lib.axon_stop_nrt_profile(str(output_dir).encode())
            print(f"ntff profile: {n} file(s) written to {output_dir}")

    mod = types.ModuleType("antenv.axon_hooks")
    mod.get_axon_ntff_profile_hook = lambda: _hook
    mod.set_axon_ntff_profile_hook = lambda h: None
    sys.modules["antenv.axon_hooks"] = mod


def _run(x, weight, trace=False):
    from concourse.bass_utils import run_bass_kernel_spmd

    if trace:
        _install_axon_ntff_hook()
    nc = _get_nc()
    in_maps = _prep_inputs(x, weight)
    res = run_bass_kernel_spmd(
        nc, in_maps, core_ids=list(range(N_CORES)), trace=trace
    )
    return _unpack_output(res), res


def kernel(x, weight):
    out, _ = _run(x, weight, trace=False)
    return out


# revision 36
# speedup vs baseline: 1.1823x; 1.0443x over previous
# Binary linear: y[b,s,o] = sum_i x[b,s,i] * sign(W)[o,i]
#
# Strategy (8 NeuronCores, data-parallel over tokens):
#   - Host: flatten x to [32768, 768] and shard 8 x [4096, 768]. Per core,
#     pack x per 512-token group with the contraction dim on SBUF
#     partitions, p-major so every DMA lands with multi-KB contiguous
#     partition rows (small rows halve the DMA queues' effective rate).
#     Contraction blocks k0/k1 are quantized to fp8 e4m3, k2..k5 stay bf16.
#     Weights are sign(W) (exactly +-1): fp8 for k0/k1, bf16 for k2..k5.
#   - Device (per core): out[o-block, token] layout. Per (group, out-slab):
#     four bf16 matmuls (k2..k5, N=512) plus ONE fp8 DoubleRow matmul that
#     contracts k0+k1 together at ~1.44x the bf16 rate. Out-slabs run in
#     PAIRS with the k-loop interleaved between the two PSUM banks: a
#     single bank caps the accumulate stream at ~2.0GHz, alternating banks
#     sustains the full ~2.4GHz PE rate. Evictions are DVE f32->bf16 casts
#     (kept off the scalar engine, whose DMA issues would delay them and
#     stall PSUM recycling); y stores are linear 128KB DMAs balanced across
#     the two hardware DMA queues. A short PE warmup covers the ~3us DMA
#     launch+ramp latency.
#   - Accuracy: only x carries rounding error (w is exact): fp8 on 2/6 of
#     the contraction + bf16 elsewhere + bf16 y => rel err ~1.55e-2
#     (measured), within the 2e-2 gate with margin.
#   - Host: unpack [os][g][128, 512] -> [4, 8192, 768] f32.

import numpy as np

N_CORES = 8
B, S, D_IN, D_OUT = 4, 8192, 768, 768
T_TOTAL = B * S              # 32768 tokens
T_CORE = T_TOTAL // N_CORES  # 4096 tokens per core
P = 128
KB = D_IN // P               # 6 contraction blocks (k0/k1 fp8, k2-5 bf16)
OS = D_OUT // P              # 6 out-feature slabs
TG = 512                     # tokens per group (one PSUM bank of f32)
G = T_CORE // TG             # 8 groups per core
N_WARMUP = 5

_cache = {}


def _build():
    import concourse.bacc as bacc
    import concourse.mybir as mybir
    import concourse.tile as tile

    f32 = mybir.dt.float32
    bf16 = mybir.dt.bfloat16
    fp8 = mybir.dt.float8e4
    DR = mybir.MatmulPerfMode.DoubleRowSwInterleave

    nc = bacc.Bacc(
        "TRN2",
        target_bir_lowering=False,
        debug=False,
        num_devices=N_CORES,
    )

    # fp8 x pair-interleaved per token ([t, pair] innermost) so the DR
    # moving stream presents both pair values adjacently per column
    x8P = nc.dram_tensor("x8P", [G, P, TG, 2], fp8, kind="ExternalInput")
    xBP = nc.dram_tensor("xBP", [G, P, KB - 2, TG], bf16, kind="ExternalInput")
    w8P = nc.dram_tensor("w8P", [P, OS, 2, P], fp8, kind="ExternalInput")
    wBP = nc.dram_tensor("wBP", [OS, P, KB - 2, P], bf16, kind="ExternalInput")
    yP = nc.dram_tensor("yP", [OS, G, P, TG], bf16, kind="ExternalOutput")

    with tile.TileContext(nc) as tc:
        with (
            tc.tile_pool(name="wpool", bufs=1) as w_pool,
            tc.tile_pool(name="xpool", bufs=1) as x_pool,
            tc.tile_pool(name="ypool", bufs=8) as y_pool,
            tc.tile_pool(name="psum", bufs=7, space="PSUM") as psum_pool,
        ):
            # --- PE warmup: dummy matmuls on zeroed scratch so the PE clock
            # has ramped by the time the first real operands land. ---
            wu = x_pool.tile([P, P + TG], bf16, tag="wu", name="wu", bufs=1)
            nc.gpsimd.memset(wu[:], 0.0)
            wups = psum_pool.tile([P, TG], f32, tag="wups", name="wups", bufs=1)
            for _ in range(N_WARMUP):
                nc.tensor.matmul(
                    wups[:], wu[:, :P], wu[:, P:],
                    start=True, stop=True, skip_group_check=True,
                )
            wu_out = x_pool.tile([P, TG], bf16, tag="wuo", name="wuo", bufs=1)
            nc.vector.tensor_copy(wu_out[:], wups[:])

            # --- loads, interleaved across the two HW queues in need order
            # (the per-(g,os) k-order is bf16 k2..k5 first, fp8 DR last, so
            # the fp8 operands may arrive ~1.7us later than the bf16 ones)
            w8all = w_pool.tile([P, OS, 2, P], fp8, tag="w8", name="w8")
            wB = [None] * OS

            def wB_load(os_, eng):
                t = w_pool.tile([P, KB - 2, P], bf16, tag=f"wB{os_}", name=f"wB{os_}")
                eng.dma_start(t[:], wBP[os_])
                wB[os_] = t

            x8 = [None] * G

            def x8_load(g, eng):
                t = x_pool.tile([P, TG, 2], fp8, tag=f"x8_{g}", name=f"x8_{g}")
                eng.dma_start(t[:], x8P[g])
                x8[g] = t

            xB = [None] * G
            xB0h = [None, None]

            def xB0_load(h, eng):
                # group 0 bf16 x as two k-pair tiles so the queues can fill
                # them in parallel just ahead of the PE
                t = x_pool.tile([P, 2, TG], bf16, tag=f"xB0_{h}", name=f"xB0_{h}")
                eng.dma_start(t[:], xBP[0, :, 2 * h : 2 * h + 2, :])
                xB0h[h] = t

            def xB_load(g, eng):
                t = x_pool.tile([P, KB - 2, TG], bf16, tag=f"xB{g}", name=f"xB{g}")
                eng.dma_start(t[:], xBP[g])
                xB[g] = t

            wB_load(0, nc.sync)
            nc.scalar.dma_start(w8all[:], w8P[:])
            xB0_load(0, nc.sync)
            wB_load(1, nc.scalar)
            wB_load(2, nc.sync)
            xB0_load(1, nc.scalar)
            x8_load(0, nc.scalar)
            xB_load(1, nc.sync)
            wB_load(3, nc.scalar)
            wB_load(4, nc.scalar)
            wB_load(5, nc.scalar)
            x8_load(1, nc.scalar)
            xB_load(3, nc.sync)
            xB_load(2, nc.scalar)
            x8_load(2, nc.scalar)
            x8_load(3, nc.scalar)
            xB_load(5, nc.sync)
            xB_load(4, nc.scalar)
            x8_load(4, nc.scalar)
            x8_load(5, nc.scalar)
            xB_load(6, nc.sync)
            x8_load(6, nc.scalar)
            x8_load(7, nc.scalar)
            xB_load(7, nc.sync)

            def rhsB(g, k):
                if g == 0:
                    return xB0h[(k - 2) // 2][:, (k - 2) % 2, :]
                return xB[g][:, k - 2, :]

            # --- main loop: out-slab pairs, k-loop interleaved across the
            # pair's two PSUM banks to sustain the full PE rate; bf16 k2..k5
            # first, then one fp8 DoubleRow matmul contracting k0+k1 ---
            ecnt = 0
            for g in range(G):
                for osp in range(OS // 2):
                    os_a, os_b = 2 * osp, 2 * osp + 1
                    ps_a = psum_pool.tile([P, TG], f32, tag="ps", name=f"ps{g}_{os_a}")
                    ps_b = psum_pool.tile([P, TG], f32, tag="ps", name=f"ps{g}_{os_b}")
                    for k in range(2, KB):
                        st = k == 2
                        nc.tensor.matmul(
                            ps_a[:], wB[os_a][:, k - 2, :], rhsB(g, k),
                            start=st, stop=False,
                        )
                        nc.tensor.matmul(
                            ps_b[:], wB[os_b][:, k - 2, :], rhsB(g, k),
                            start=st, stop=False,
                        )
                    rhs8 = x8[g][:].rearrange("p t two -> p two t")
                    nc.tensor.matmul(
                        ps_a[:], w8all[:, os_a, :, :], rhs8,
                        start=False, stop=True, perf_mode=DR,
                    )
                    nc.tensor.matmul(
                        ps_b[:], w8all[:, os_b, :, :], rhs8,
                        start=False, stop=True, perf_mode=DR,
                    )
                    for os_, ps in ((os_a, ps_a), (os_b, ps_b)):
                        yt = y_pool.tile([P, TG], bf16, tag="y", name=f"y{g}_{os_}")
                        if g == G - 1 and osp == OS // 2 - 1:
                            # tail pair: halves in parallel on both copy
                            # engines and both DMA queues so the final
                            # receipts land as early as possible
                            h = TG // 2
                            nc.vector.tensor_copy(yt[:, :h], ps[:, :h])
                            nc.scalar.copy(yt[:, h:], ps[:, h:])
                            nc.sync.dma_start(yP[os_, g, :, :h], yt[:, :h])
                            nc.scalar.dma_start(yP[os_, g, :, h:], yt[:, h:])
                            ecnt += 1
                            continue
                        # all evictions on DVE: the scalar engine's DMA
                        # issues would delay them and stall PSUM recycling
                        nc.vector.tensor_copy(yt[:], ps[:])
                        # store queues: scalar while sync still streams x,
                        # alternating afterwards so neither queue backs up
                        # near the tail
                        if g <= 4:
                            q = nc.scalar
                        else:
                            q = nc.sync if os_ % 2 == 0 else nc.scalar
                        q.dma_start(yP[os_, g], yt[:])
                        ecnt += 1

    nc.compile()
    return nc


def _get_nc():
    if "nc" not in _cache:
        _cache["nc"] = _build()
    return _cache["nc"]


def _prep_inputs(x, weight):
    import ml_dtypes

    x = np.asarray(x, dtype=np.float32)
    w = np.asarray(weight, dtype=np.float32)
    x2 = x.reshape(N_CORES, T_CORE, D_IN)
    # x5[c, g, t, k, p] = x2[c, g*TG + t, k*P + p] -> packs [c, g, p, k, t]
    x5 = x2.reshape(N_CORES, G, TG, KB, P)
    # [c, g, p, t, pair]: both pair values adjacent per token column
    x8Pack = np.ascontiguousarray(x5[:, :, :, :2, :].transpose(0, 1, 4, 2, 3)).astype(
        ml_dtypes.float8_e4m3fn
    )
    xBPack = np.ascontiguousarray(x5[:, :, :, 2:, :].transpose(0, 1, 4, 3, 2)).astype(
        ml_dtypes.bfloat16
    )
    # S4[os, o, k, p] = sign(W)[os*P + o, k*P + p]  (+-1/0 exact in both)
    S4 = np.sign(w).reshape(OS, P, KB, P)
    # SwInterleave weight layout: per partition row the k0/k1 weights are
    # interleaved pairwise with output columns reversed:
    # [A127, B127, A126, B126, ..., A0, B0]  (A=k0, B=k1)
    w8i = S4[:, ::-1, :2, :].transpose(3, 0, 1, 2)  # [p, os, o_rev, k(2)]
    w8Pack = np.ascontiguousarray(
        w8i.reshape(P, OS, 2, P)
    ).astype(ml_dtypes.float8_e4m3fn)
    wBPack = np.ascontiguousarray(S4[:, :, 2:, :].transpose(0, 3, 2, 1)).astype(
        ml_dtypes.bfloat16
    )
    return [
        {"x8P": x8Pack[c], "xBP": xBPack[c], "w8P": w8Pack, "wBP": wBPack}
        for c in range(N_CORES)
    ]


def _unpack_output(res):
    # yP [OS, G, P(o), TG(t)] -> y_core [T_CORE, D_OUT]
    outs = []
    for r in res.results:
        yp = np.asarray(r["yP"]).astype(np.float32)
        outs.append(yp.transpose(1, 3, 0, 2).reshape(T_CORE, D_OUT))
    return np.concatenate(outs, axis=0).reshape(B, S, D_OUT)


def _install_axon_ntff_hook():
    """The agent image's `antenv` lacks `axon_hooks`; register an equivalent
    module backed by direct ctypes calls into libaxon_pjrt.so so that
    run_bass_kernel_spmd(trace=True) can capture NTFF profiles under axon."""
    import sys

    if "antenv.axon_hooks" in sys.modules:
        return
    import contextlib
    import ctypes
    import types

    so_path = "/opt/axon/libaxon_pjrt.so"
    try:
        lib = ctypes.CDLL(so_path)
    except OSError:
        return
    if not hasattr(lib, "axon_start_nrt_profile"):
        return
    lib.axon_start_nrt_profile.argtypes = [
        ctypes.POINTER(ctypes.c_int64),
        ctypes.c_size_t,
    ]
    lib.axon_start_nrt_profile.restype = ctypes.c_int64
    lib.axon_stop_nrt_profile.argtypes = [ctypes.c_char_p]
    lib.axon_stop_nrt_profile.restype = ctypes.c_int64

    @contextlib.contextmanager
    def _hook(output_dir, device_ids):
        import jax

        jax.devices()
        if device_ids:
            ids = (ctypes.c_int64 * len(device_ids))(*device_ids)
            rc = lib.axon_start_nrt_profile(ids, len(device_ids))
        else:
            rc = lib.axon_start_nrt_profile(None, 0)
        if rc != 0:
            raise RuntimeError(f"axon_start_nrt_profile rc={rc}")
        try:
            yield
        finally:
            n = lib.axon_stop_nrt_profile(str(output_dir).encode())
            print(f"ntff profile: {n} file(s) written to {output_dir}")

    mod = types.ModuleType("antenv.axon_hooks")
    mod.get_axon_ntff_profile_hook = lambda: _hook
    mod.set_axon_ntff_profile_hook = lambda h: None
    sys.modules["antenv.axon_hooks"] = mod


def _run(x, weight, trace=False):
    from concourse.bass_utils import run_bass_kernel_spmd

    if trace:
        _install_axon_ntff_hook()
    nc = _get_nc()
    in_maps = _prep_inputs(x, weight)
    res = run_bass_kernel_spmd(
        nc, in_maps, core_ids=list(range(N_CORES)), trace=trace
    )
    return _unpack_output(res), res


def kernel(x, weight):
    out, _ = _run(x, weight, trace=False)
    return out
